# revision 14
# baseline (speedup 1.0000x reference)
"""Trainium2 Bass kernel for nn_AutoencoderInverseAffine.

out[n] = (samples[n] - mus_[symb[n], comp[n]]) / psi_c[comp[n]] + mus_orig_[symb[n], comp[n]]
       = samples[n] * A[comp[n]] + B[symb[n]*4 + comp[n]]

with A = 1/psi (4x8) and B = mus_orig - mus * A (64x8) precomputed on host
(tiny parameter tables). The 8M rows are data-parallel across the 8
NeuronCores; on-device each row's two table vectors are materialized via
per-class is_equal masks + predicated copies on the Vector engine, then a
fused multiply-add produces the output.
"""

import os
import numpy as np

import concourse.bass as bass
import concourse.bacc as bacc
import concourse.mybir as mybir
import concourse.tile as tile
from concourse.bass_utils import run_bass_kernel_spmd
from contextlib import ExitStack

F32 = mybir.dt.float32
BF16 = mybir.dt.bfloat16
U32 = mybir.dt.uint32

N_SAMP = 8388608
N_DIM = 8
NX = 16
N_COMP = 4
N_CLASS = NX * N_COMP  # 64
NCORES = 8
R = N_SAMP // NCORES   # rows per core
C = 512                # rows per partition per tile
NT = R // (128 * C)    # tiles per core

_cache = {}


def _build():
    nc = bacc.Bacc("TRN2", target_bir_lowering=False, debug=False,
                   num_devices=NCORES)
    samp = nc.dram_tensor("samples", (R, N_DIM), BF16, kind="ExternalInput").ap()
    jidx = nc.dram_tensor("jidx", (R,), BF16, kind="ExternalInput").ap()
    cidx = nc.dram_tensor("cidx", (R,), BF16, kind="ExternalInput").ap()
    tabd = nc.dram_tensor("tab", (128, (N_COMP + N_CLASS) * N_DIM), BF16,
                          kind="ExternalInput").ap()
    outd = nc.dram_tensor("out", (R, N_DIM), BF16, kind="ExternalOutput").ap()

    s3 = samp.rearrange("(t p c) d -> t p (c d)", p=128, c=C)
    o3 = outd.rearrange("(t p c) d -> t p (c d)", p=128, c=C)
    j3 = jidx.rearrange("(t p c) -> t p c", p=128, c=C)
    c3 = cidx.rearrange("(t p c) -> t p c", p=128, c=C)

    with tile.TileContext(nc) as tc, ExitStack() as ctx:
        consts = ctx.enter_context(tc.tile_pool(name="consts", bufs=1))
        io = ctx.enter_context(tc.tile_pool(name="io", bufs=2))
        work = ctx.enter_context(tc.tile_pool(name="work", bufs=1))
        outp = ctx.enter_context(tc.tile_pool(name="outp", bufs=2))

        tab = consts.tile([128, (N_COMP + N_CLASS) * N_DIM], BF16)
        nc.gpsimd.dma_start(tab[:], tabd[:])

        def tab_vec(k):
            # class-k 8-bf16 vector, bitcast to 4 u32, broadcast to (128, C, 4)
            v = tab[:, 8 * k:8 * k + 8].bitcast(U32)
            return v.unsqueeze(1).broadcast_to([128, C, N_DIM // 2])

        for t in range(NT):
            st = io.tile([128, C * N_DIM], BF16, tag="samp")
            nc.gpsimd.dma_start(st[:], s3[t])
            jt = io.tile([128, C], BF16, tag="jidx")
            nc.gpsimd.dma_start(jt[:], j3[t])
            ct = io.tile([128, C], BF16, tag="cidx")
            nc.gpsimd.dma_start(ct[:], c3[t])

            gA = work.tile([128, C * N_DIM], BF16, tag="gA")
            gB = work.tile([128, C * N_DIM], BF16, tag="gB")
            mask = work.tile([128, C], mybir.dt.uint8, tag="mask")
            gA3 = gA[:].bitcast(U32).rearrange("p (c d) -> p c d", d=N_DIM // 2)
            gB3 = gB[:].bitcast(U32).rearrange("p (c d) -> p c d", d=N_DIM // 2)
            mask3 = mask[:].unsqueeze(2).broadcast_to([128, C, N_DIM // 2])

            for k in range(N_COMP):
                nc.vector.tensor_scalar(mask[:], ct[:], float(k), None,
                                        mybir.AluOpType.is_equal)
                nc.vector.copy_predicated(gA3, mask3, tab_vec(k))
            for k in range(N_CLASS):
                nc.vector.tensor_scalar(mask[:], jt[:], float(k), None,
                                        mybir.AluOpType.is_equal)
                nc.vector.copy_predicated(gB3, mask3, tab_vec(N_COMP + k))

            prod = work.tile([128, C * N_DIM], BF16, tag="prod")
            nc.vector.tensor_mul(prod[:], st[:], gA[:])
            ot = outp.tile([128, C * N_DIM], BF16, tag="out")
            nc.vector.tensor_add(ot[:], prod[:], gB[:])
            nc.gpsimd.dma_start(o3[t], ot[:])

    nc.compile()
    return nc


def _host_prep(samples_, mus_orig_, mus_, psi_c_, idx_symb_, idx_comp_):
    import ml_dtypes
    bf16 = ml_dtypes.bfloat16
    A = (1.0 / psi_c_.reshape(N_COMP, N_DIM)).astype(np.float32)
    mu3 = np.asarray(mus_).reshape(NX, N_COMP, N_DIM).astype(np.float32)
    mo3 = np.asarray(mus_orig_).reshape(NX, N_COMP, N_DIM).astype(np.float32)
    B = (mo3 - mu3 * A[None]).reshape(N_CLASS, N_DIM).astype(np.float32)
    tab1 = np.concatenate([A.reshape(-1), B.reshape(-1)]).astype(bf16)
    tab = np.broadcast_to(tab1, (128, tab1.size)).copy()

    j = (np.asarray(idx_symb_, dtype=np.int64) * N_COMP
         + np.asarray(idx_comp_, dtype=np.int64)).astype(bf16)
    c = np.asarray(idx_comp_, dtype=np.float32).astype(bf16)
    samples = np.ascontiguousarray(np.asarray(samples_, dtype=np.float32)).astype(bf16)
    return samples, j, c, tab


def kernel(samples_, mus_orig_, mus_, psi_c_, idx_symb_, idx_comp_,
           n_samp_=None, n_dim_=None, **_unused):
    samples, j, c, tab = _host_prep(np.asarray(samples_), np.asarray(mus_orig_),
                                    np.asarray(mus_), np.asarray(psi_c_),
                                    np.asarray(idx_symb_), np.asarray(idx_comp_))
    if "nc" not in _cache:
        _cache["nc"] = _build()
    nc = _cache["nc"]

    in_maps = []
    for i in range(NCORES):
        sl = slice(i * R, (i + 1) * R)
        in_maps.append({
            "samples": samples[sl],
            "jidx": j[sl],
            "cidx": c[sl],
            "tab": tab,
        })

    trace = bool(os.environ.get("KERNEL_TRACE"))
    kwargs = {}
    if trace:
        # antenv.axon_hooks is missing in this image; shim it so trace works.
        import sys
        import types
        if "antenv.axon_hooks" not in sys.modules:
            import trn_agent_boot.trn_boot as _tb
            m = types.ModuleType("antenv.axon_hooks")
            holder = [None]
            m.set_axon_ntff_profile_hook = lambda h: holder.__setitem__(0, h)
            m.get_axon_ntff_profile_hook = lambda: holder[0]
            sys.modules["antenv.axon_hooks"] = m
            m.set_axon_ntff_profile_hook(
                _tb._ntff_profile_via_ctypes("/opt/axon/libaxon_pjrt.so"))
        kwargs = {"trace": True,
                  "tmpdir": os.environ.get("KERNEL_TRACE_DIR") or None}

    res = run_bass_kernel_spmd(nc, in_maps, core_ids=list(range(NCORES)), **kwargs)
    if trace:
        _cache["exec_time_ns"] = res.exec_time_ns
        _cache["profile_json"] = res.profile_json

    out = np.concatenate([res.results[i]["out"] for i in range(NCORES)], axis=0)
    return out.astype(np.float32)


# revision 17
# speedup vs baseline: 1.7727x; 1.7727x over previous
"""Trainium2 Bass kernel for nn_AutoencoderInverseAffine.

out[n] = (samples[n] - mus_[s_n, c_n]) / psi_c[c_n] + mus_orig_[s_n, c_n]
       = samples[n] * Atilde[j_n] + B[j_n],   j_n = 4*s_n + c_n

Atilde = tile(1/psi, 16) and B = mus_orig - mus/psi are tiny 64x8 tables
precomputed on host. Rows are data-parallel across the 8 NeuronCores.

On-device per 512-pair block (1024 rows):
 1. jbcast matmul (K=2, row-strip 32*t4): broadcasts the block's even/odd
    row indices jE/jO to 64+64 partitions of a PSUM bank.
 2. DVE is_equal vs a per-partition iota (p%64) builds the stacked one-hot
    pair (128, 512) in bf16.
 3. gather matmul (K=128, M=32, col-strip 32*t4): one-hot @ [Atilde;B]
    yields each pair's [eA8 eB8 oA8 oB8] in a PSUM bank strip.
 4. The staged (128, 512) bank is xbar DMA-transposed in (128, 128)
    chunks (the only SBUF->SBUF shape the xbar handles correctly:
    dest[p,x] = src[x,p]) into a row-major-strided layout.
 5. One strided-4-dim-AP multiply + add per tile: out = samples*A + B.

All data moves in bfloat16 (inputs converted on host), which halves HBM
traffic; l2 relative error ~3e-3 vs the f32 reference.
"""

import os
import numpy as np
import ml_dtypes

import concourse.bacc as bacc
import concourse.mybir as mybir
import concourse.tile as tile
from concourse.bass_utils import run_bass_kernel_spmd
from contextlib import ExitStack

F32 = mybir.dt.float32
BF16 = mybir.dt.bfloat16
bf16 = ml_dtypes.bfloat16

N_SAMP = 8388608
N_DIM = 8
NX = 16
N_COMP = 4
N_CLASS = 64
NCORES = 8
R = N_SAMP // NCORES   # 1048576 rows per core
C = 512                # rows per partition per tile
TILE_ROWS = 128 * C    # 65536
NT = R // TILE_ROWS    # 16 tiles per core

_cache = {}


def _build_tables(mus_orig_, mus_, psi_c_):
    A = (1.0 / np.asarray(psi_c_, np.float32).reshape(N_COMP, N_DIM))
    mu3 = np.asarray(mus_, np.float32).reshape(NX, N_COMP, N_DIM)
    mo3 = np.asarray(mus_orig_, np.float32).reshape(NX, N_COMP, N_DIM)
    B = (mo3 - mu3 * A[None]).reshape(N_CLASS, N_DIM)
    At = np.tile(A, (NX, 1))

    wtg = np.zeros((128, 32), np.float32)
    wtg[:64, 0:8] = At
    wtg[:64, 8:16] = B
    wtg[64:, 16:24] = At
    wtg[64:, 24:32] = B

    wt2 = np.zeros((128, 128), np.float32)
    for t4 in range(4):
        wt2[32 * t4 + 0, :64] = 1.0
        wt2[32 * t4 + 1, 64:] = 1.0

    iota = (np.arange(128, dtype=np.float32) % 64).reshape(128, 1)
    return wtg.astype(bf16), wt2.astype(bf16), iota


def _prep_j(j_core, ntiles):
    """j (R,) int -> (ntiles, 8, 8192) bf16; row 2*t4+e holds strip t4's
    jE/jO stream in (G, r4, k4, p) order."""
    out = np.empty((ntiles, 8, 8192), dtype=bf16)
    for t in range(ntiles):
        jj = j_core[t * TILE_ROWS:(t + 1) * TILE_ROWS].astype(np.float32)
        jm = jj.reshape(128, 16, 4, 4, 2)  # p, r, f, t4, e ; pair m = 16r+4f+t4
        out[t] = jm.transpose(3, 4, 1, 2, 0).reshape(8, 8192).astype(bf16)
    return out


def _build_nc():
    nc = bacc.Bacc("TRN2", target_bir_lowering=False, debug=False,
                   num_devices=NCORES)
    samp = nc.dram_tensor("samples", (R, N_DIM), BF16, kind="ExternalInput").ap()
    jrd = nc.dram_tensor("jrows", (NT, 8, 8192), BF16, kind="ExternalInput").ap()
    wtgd = nc.dram_tensor("wtg", (128, 32), BF16, kind="ExternalInput").ap()
    wt2d = nc.dram_tensor("wt2", (128, 128), BF16, kind="ExternalInput").ap()
    iotad = nc.dram_tensor("iota", (128, 1), F32, kind="ExternalInput").ap()
    outd = nc.dram_tensor("out", (R, N_DIM), BF16, kind="ExternalOutput").ap()

    s3 = samp.rearrange("(t p c) d -> t p (c d)", p=128, c=C)
    o3 = outd.rearrange("(t p c) d -> t p (c d)", p=128, c=C)

    with tile.TileContext(nc) as tc, ExitStack() as ctx:
        consts = ctx.enter_context(tc.tile_pool(name="consts", bufs=1))
        iop = ctx.enter_context(tc.tile_pool(name="iop", bufs=2))
        jrp = ctx.enter_context(tc.tile_pool(name="jrp", bufs=2))
        ohp = ctx.enter_context(tc.tile_pool(name="ohp", bufs=8))
        gsbp = ctx.enter_context(tc.tile_pool(name="gsbp", bufs=4))
        grmp = ctx.enter_context(tc.tile_pool(name="grmp", bufs=3))
        outp = ctx.enter_context(tc.tile_pool(name="outp", bufs=2))
        jbp = ctx.enter_context(tc.tile_pool(name="jbp", bufs=4, space="PSUM"))
        gp = ctx.enter_context(tc.tile_pool(name="gp", bufs=2, space="PSUM"))

        wtg = consts.tile([128, 32], BF16)
        nc.gpsimd.dma_start(wtg[:], wtgd[:])
        wt2 = consts.tile([128, 128], BF16)
        nc.gpsimd.dma_start(wt2[:], wt2d[:])
        iota = consts.tile([128, 1], F32)
        nc.gpsimd.dma_start(iota[:], iotad[:])

        for t in range(NT):
            st = iop.tile([128, C * N_DIM], BF16, tag="samp")
            nc.gpsimd.dma_start(st[:], s3[t])
            jr = jrp.tile([128, 8192], BF16, tag="jr")
            for t4 in range(4):
                nc.gpsimd.dma_start(jr[32 * t4:32 * t4 + 2, :],
                                    jrd[t, 2 * t4:2 * t4 + 2, :])

            grm = grmp.tile([128, C * 16], BF16, tag="grm")

            for r in range(16):
                g = gp.tile([128, 512], F32, tag="g")
                for t4 in range(4):
                    blk = r * 512
                    jb = jbp.tile([128, 512], F32, tag="jb")
                    nc.tensor.matmul(jb[:],
                                     wt2[32 * t4:32 * t4 + 2, :],
                                     jr[32 * t4:32 * t4 + 2, blk:blk + 512],
                                     start=True, stop=True,
                                     tile_position=(32 * t4, 0))
                    oh = ohp.tile([128, 512], BF16, tag="oh")
                    nc.vector.tensor_scalar(oh[:], jb[:], iota[:], None,
                                            mybir.AluOpType.is_equal)
                    nc.tensor.matmul(g[32 * t4:32 * t4 + 32, :],
                                     wtg[:], oh[:],
                                     start=True, stop=True,
                                     tile_position=(0, 32 * t4))
                gsb = gsbp.tile([128, 512], BF16, tag="gsb")
                nc.vector.tensor_copy(gsb[:], g[:])
                for f in range(4):
                    dst = grm[:, (r * 4 + f) * 128:(r * 4 + f) * 128 + 128]
                    nc.sync.dma_start_transpose(dst, gsb[:, f * 128:f * 128 + 128])

            # dest[p, x] = src[x, p] per (128,128) chunk, so
            # grm offset = 32*w + 16*e + 8*ab + d with pair m = w = 16r+4f+t4
            # st  offset = 16*w + 8*e + d
            stv = st[:].rearrange("p (w e d) -> p w e d", w=256, e=2, d=8)
            gv = grm[:].rearrange("p (w e ab d) -> p w e ab d",
                                  w=256, e=2, ab=2, d=8)
            prod = outp.tile([128, C * N_DIM], BF16, tag="prod")
            ot = outp.tile([128, C * N_DIM], BF16, tag="out")
            pv = prod[:].rearrange("p (w e d) -> p w e d", w=256, e=2, d=8)
            ov = ot[:].rearrange("p (w e d) -> p w e d", w=256, e=2, d=8)
            for e in range(2):
                nc.vector.tensor_mul(pv[:, :, e, :], stv[:, :, e, :],
                                     gv[:, :, e, 0, :])
                nc.vector.tensor_add(ov[:, :, e, :], pv[:, :, e, :],
                                     gv[:, :, e, 1, :])
            nc.gpsimd.dma_start(o3[t], ot[:])

    nc.compile()
    return nc


def kernel(samples_, mus_orig_, mus_, psi_c_, idx_symb_, idx_comp_,
           n_samp_=None, n_dim_=None, **_unused):
    wtg, wt2, iota = _build_tables(np.asarray(mus_orig_), np.asarray(mus_),
                                   np.asarray(psi_c_))
    j = (np.asarray(idx_symb_, dtype=np.int64) * N_COMP
         + np.asarray(idx_comp_, dtype=np.int64))
    samples = np.ascontiguousarray(
        np.asarray(samples_, dtype=np.float32)).astype(bf16)

    if "nc" not in _cache:
        _cache["nc"] = _build_nc()
    nc = _cache["nc"]

    in_maps = []
    for i in range(NCORES):
        sl = slice(i * R, (i + 1) * R)
        in_maps.append({
            "samples": samples[sl],
            "jrows": _prep_j(j[sl], NT),
            "wtg": wtg,
            "wt2": wt2,
            "iota": iota,
        })

    trace = bool(os.environ.get("KERNEL_TRACE"))
    kwargs = {}
    if trace:
        # antenv.axon_hooks is missing in this image; shim it so trace works.
        import sys
        import types
        if "antenv.axon_hooks" not in sys.modules:
            import trn_agent_boot.trn_boot as _tb
            m = types.ModuleType("antenv.axon_hooks")
            holder = [None]
            m.set_axon_ntff_profile_hook = lambda h: holder.__setitem__(0, h)
            m.get_axon_ntff_profile_hook = lambda: holder[0]
            sys.modules["antenv.axon_hooks"] = m
            m.set_axon_ntff_profile_hook(
                _tb._ntff_profile_via_ctypes("/opt/axon/libaxon_pjrt.so"))
        kwargs = {"trace": True,
                  "tmpdir": os.environ.get("KERNEL_TRACE_DIR") or None}

    res = run_bass_kernel_spmd(nc, in_maps, core_ids=list(range(NCORES)), **kwargs)
    if trace:
        _cache["exec_time_ns"] = res.exec_time_ns
        _cache["profile_json"] = res.profile_json

    out = np.concatenate([res.results[i]["out"] for i in range(NCORES)], axis=0)
    return out.astype(np.float32)
